# revision 7
# baseline (speedup 1.0000x reference)
"""ChannelMerger kernel for 8x Trainium2 NeuronCores (Bass/Tile).

Computes, for eeg [B,T,C], positions [B,C,2], heads [O,D]:
    emb     = fourier_emb(positions)              # [B,C,D], D = 2*12*12
    scores  = einsum('bcd,od->boc', emb, heads)   # [B,O,C]
    weights = softmax(scores, axis=2)
    out     = einsum('bct,boc->bot', eeg_ct, weights).transpose -> [B,T,O]

Sharding: data-parallel over batch B=32 -> 4 batches per core on 8 cores.

Layout strategy: the host hands each core eeg already transposed to
[BPC, C, T] and cast to bf16 (layout/precision transform only; all FLOPs
run on device).  With the contraction dim C on partitions, the big einsum
is a single stationary-weights matmul stream per batch:
    outT[o, t] = sum_c wT[c, o] * eegT[c, t]
so the PE streams T columns at 1 cycle/column (bf16) with zero on-chip
transposes of eeg.  Output is written as [BPC, O, T] bf16 and the host
transposes back to [B, T, O] f32.  HBM traffic per core: 8 MB in + 4 MB
out; the DMA rings (16 engines x ~24 GB/s) are the roofline (~35 us).

v3 over v2 (59.7us):
 - positions go up as one 4KB row, replicated on-chip by gpsimd
   partition_broadcast (saves a 0.5MB HBM read on the critical path)
 - fourier phase chains for freq-chunk 0/1 run on vector/gpsimd in
   parallel; embq+ht4 are bf16 so scores are 4 wide matmuls
 - per-batch 2MB eeg DMAs (fewer serial DMA dispatches on Sync)
 - PSUM pair-packing: group g -> partitions 0:64, group g+4 -> 64:128 of
   the same bank, so PSUM->SBUF copies move 128 partitions per op; copies
   round-robin over scalar/vector/gpsimd
"""

import numpy as np
import ml_dtypes

import concourse.bacc as bacc
import concourse.mybir as mybir
import concourse.tile as tile

# ---------------------------------------------------------------- constants
B, T, C = 32, 8192, 128
O = 64
N_FREQS = 12
N_IJ = N_FREQS * N_FREQS          # 144
D = 2 * N_IJ                      # 288
MARGIN = 0.2
N_CORES = 8
BPC = B // N_CORES                # batches per core = 4
NGRP = 16                        # matmul groups per batch (n=512 each)
TGRP = T // NGRP                  # 512
HGRP = NGRP // 2                  # groups per half batch = 8
QUAD = 4                          # pair-packed tiles per half
F32 = mybir.dt.float32
BF16 = mybir.dt.bfloat16
BF16_NP = ml_dtypes.bfloat16


# ------------------------------------------------------------ host constants
def _host_constants(heads: np.ndarray):
    """Pure layout/padding transforms of `heads` + static tables."""
    width = 1.0 + 2.0 * MARGIN
    # Frequencies in TURNS (cycles): loc_rad = 2*pi * (pos_x*p_i + pos_y*p_j).
    # Working in turns lets the device reduce the phase into [-pi, pi] with a
    # round-to-nearest int cast before the Sin table lookup.
    p = np.arange(N_FREQS, dtype=np.float64) / width

    # misc [128, 68] f32: cols 0:4 per-partition frequency pairs for the two
    # ij chunks (chunk c covers ij = 128c + k, k = partition; entries past
    # 143 are 0 and their heads rows are zero-padded), cols 4:68 a 64x64
    # identity on partitions 0:64 for the weight transpose.
    misc = np.zeros((128, 68), dtype=np.float32)
    for c in range(2):
        for k in range(128):
            ij = 128 * c + k
            if ij < N_IJ:
                misc[k, 2 * c + 0] = p[ij // N_FREQS]
                misc[k, 2 * c + 1] = p[ij % N_FREQS]
    misc[0:O, 4:4 + O] = np.eye(O, dtype=np.float32)

    # headsT chunks [K=128, O] for the 4 embT chunks (cos0, cos1, sin0, sin1)
    ht4 = np.zeros((128, 4 * O), dtype=np.float32)
    ht4[:, 0 * O:1 * O] = heads[:, 0:128].T               # cos ij 0..127
    ht4[:16, 1 * O:2 * O] = heads[:, 128:144].T           # cos ij 128..143
    ht4[:, 2 * O:3 * O] = heads[:, 144:272].T             # sin ij 0..127
    ht4[:16, 3 * O:4 * O] = heads[:, 272:288].T           # sin ij 128..143

    return misc, ht4.astype(BF16_NP)


def _pos_row(positions_core: np.ndarray) -> np.ndarray:
    """[BPC,C,2] -> [1, BPC*256] with (x+MARGIN | y+MARGIN) per batch."""
    pos = positions_core.astype(np.float32) + np.float32(MARGIN)
    row = np.concatenate(
        [np.concatenate([pos[b, :, 0], pos[b, :, 1]]) for b in range(BPC)]
    )  # [BPC*256]
    return row.reshape(1, -1).copy()


# ------------------------------------------------------------- device kernel
def _build_nc():
    # Bacc (not plain Bass): finalize() runs generate_event_semaphores,
    # which splits multi-sem waits (TRN2 allows 1 wait per instruction).
    nc = bacc.Bacc()
    eegT = nc.declare_dram_parameter("eegT", [BPC, C, T], BF16, isOutput=False)
    posr = nc.declare_dram_parameter("posr", [1, BPC * 2 * C], F32, isOutput=False)
    ht4 = nc.declare_dram_parameter("ht4", [128, 4 * O], BF16, isOutput=False)
    misc = nc.declare_dram_parameter("misc", [128, 68], F32, isOutput=False)
    outT = nc.declare_dram_parameter("outT", [BPC, O, T], BF16, isOutput=True)

    TWO_PI = float(2.0 * np.pi)
    I32 = mybir.dt.int32

    with tile.TileContext(nc) as tc:
        with tc.tile_pool(name="consts", bufs=1) as cpool:
            # Issue all input DMAs first so the Sync engine starts feeding
            # the DMA rings the moment the preamble ends.
            posr_sb = cpool.tile([1, BPC * 2 * C], F32)
            nc.sync.dma_start(out=posr_sb, in_=posr[:, :])
            misc_sb = cpool.tile([128, 68], F32)
            nc.sync.dma_start(out=misc_sb, in_=misc[:, :])
            ht4_sb = cpool.tile([128, 4 * O], BF16)
            nc.sync.dma_start(out=ht4_sb, in_=ht4[:, :])
            pij_sb = misc_sb[:, 0:4]
            ident_sb = misc_sb[0:O, 4:4 + O]

            with (
                tc.tile_pool(name="ein", bufs=BPC) as ein,
                tc.tile_pool(name="wsb", bufs=1) as wsb,
                tc.tile_pool(name="osb", bufs=2) as osb,
            ):
                # One bulk 2MB DMA per batch: ring packets drain in dispatch
                # order, so batch 0 completes first and compute pipelines.
                e_tiles = []
                for b in range(BPC):
                    e_sb = ein.tile([128, T], BF16, tag="e", name=f"e_{b}")
                    nc.sync.dma_start(out=e_sb, in_=eegT[b][:, :])
                    e_tiles.append(e_sb)

                # Replicate the position row across all 128 partitions
                # on-chip (the fourier chain needs x on every partition to
                # pair with that partition's frequency).
                posb_sb = cpool.tile([128, BPC * 2 * C], F32)
                nc.gpsimd.partition_broadcast(posb_sb, posr_sb)

                # PE warm-up: the HAM clock gate keeps the PE at 1.2 GHz
                # until it sees ~3.4us of sustained matmul activity. Burn
                # cheap bf16 matmuls while the DMAs land so the real work
                # runs at 2.4 GHz from the start.
                wu_a = cpool.tile([128, 128], BF16)
                wu_b = cpool.tile([128, 512], BF16)
                nc.vector.memset(wu_a, 1.0)
                nc.vector.memset(wu_b, 1.0)
                with tc.tile_pool(name="wups", bufs=1, space="PSUM") as wups:
                    wu_ps = wups.tile([128, 512], F32)
                    for _ in range(20):
                        nc.tensor.matmul(out=wu_ps, lhsT=wu_a, rhs=wu_b,
                                         start=True, stop=True)

                # PSUM pools open after the warm-up pool releases its bank:
                # wps (2 tags) + opp (6) fill all 8 banks.
                with (
                    tc.tile_pool(name="wps", bufs=1, space="PSUM") as wps,
                    tc.tile_pool(name="opp", bufs=6, space="PSUM") as opp,
                ):
                    _main_phase(nc, tc, wps, opp, wsb, cpool, osb,
                                posb_sb, pij_sb, ht4_sb, ident_sb,
                                e_tiles, outT)
    nc.finalize()
    return nc


def _main_phase(nc, tc, wps, opp, wsb, cpool, osb, posb_sb, pij_sb, ht4_sb,
                ident_sb, e_tiles, outT):
                TWO_PI = float(2.0 * np.pi)
                I32 = mybir.dt.int32
                # ---------- phase 0: fourier emb + scores + softmax --------
                # Freq-chunk c=0 runs on the vector engine, c=1 on gpsimd,
                # in parallel; the Sin table lookups serialize on scalar.
                pv = posb_sb.rearrange("p (b s c) -> p b s c", b=BPC, s=2)
                x_all = pv[:, :, 0, :]   # [128, BPC, C]
                y_all = pv[:, :, 1, :]
                embq = wsb.tile([128, 4, BPC, 128], BF16, tag="embq")
                for c, eng in ((0, nc.vector), (1, nc.gpsimd)):
                    # phase in turns: t = x*p_i + y*p_j  (>= 0, < ~19)
                    t1 = wsb.tile([128, BPC, 128], F32, tag=f"t1_{c}")
                    tt = wsb.tile([128, BPC, 128], F32, tag=f"tt_{c}")
                    eng.tensor_scalar_mul(
                        out=t1, in0=x_all, scalar1=pij_sb[:, 2 * c:2 * c + 1]
                    )
                    eng.tensor_scalar_mul(
                        out=tt, in0=y_all, scalar1=pij_sb[:, 2 * c + 1:2 * c + 2]
                    )
                    eng.tensor_add(out=tt, in0=tt, in1=t1)
                    tc4 = wsb.tile([128, BPC, 128], F32, tag=f"tc4_{c}")
                    eng.tensor_scalar_add(out=tc4, in0=tt, scalar1=0.25)
                    # cos chunk (t+0.25) -> q=c, sin chunk -> q=2+c.
                    # Reduce phase via round-to-nearest-even f32->i32 cast:
                    # r = t - rne(t) in [-0.5, 0.5]; sin(2pi*t) = Sin(2pi*r).
                    for src_t, q in ((tc4, c), (tt, 2 + c)):
                        ki = wsb.tile([128, BPC, 128], I32, tag=f"ki_{c}_{q}")
                        kf = wsb.tile([128, BPC, 128], F32, tag=f"kf_{c}_{q}")
                        eng.tensor_copy(out=ki, in_=src_t)
                        eng.tensor_copy(out=kf, in_=ki)
                        rr = wsb.tile([128, BPC, 128], F32, tag=f"rr_{c}_{q}")
                        eng.tensor_sub(out=rr, in0=src_t, in1=kf)
                        nc.scalar.activation(
                            out=embq[:, q, :, :], in_=rr,
                            func=mybir.ActivationFunctionType.Sin,
                            scale=TWO_PI, bias=0.0,
                        )
                scores_ps = wps.tile([O, BPC, 128], F32, tag="scores")
                for q in range(4):
                    nc.tensor.matmul(
                        out=scores_ps,
                        lhsT=ht4_sb[:, q * O:(q + 1) * O],
                        rhs=embq[:, q, :, :],
                        start=(q == 0), stop=(q == 3),
                    )
                # scores are bounded (|s| < ~10): plain exp is fp32-safe and
                # softmax is shift-invariant, so skip the max-subtraction.
                probs = wsb.tile([O, BPC, 128], F32, tag="probs")
                ssum = wsb.tile([O, BPC], F32, tag="ssum")
                for b in range(BPC):
                    nc.scalar.activation(
                        out=probs[:, b, :], in_=scores_ps[:, b, :],
                        func=mybir.ActivationFunctionType.Exp,
                        bias=0.0, accum_out=ssum[:, b:b + 1],
                    )
                rcp = wsb.tile([O, BPC], F32, tag="rcp")
                nc.vector.reciprocal(out=rcp, in_=ssum)
                wgt = wsb.tile([O, BPC, 128], F32, tag="wgt")
                wt_ps = wps.tile([128, BPC * O], F32, tag="wtps")
                for b in range(BPC):
                    nc.vector.tensor_scalar_mul(
                        out=wgt[:, b, :], in0=probs[:, b, :],
                        scalar1=rcp[:, b:b + 1],
                    )
                    nc.tensor.transpose(
                        out=wt_ps[:, b * O:(b + 1) * O], in_=wgt[:, b, :],
                        identity=ident_sb,
                    )
                wt_bf = cpool.tile([128, BPC * O], BF16)
                nc.vector.tensor_copy(out=wt_bf, in_=wt_ps)  # f32 -> bf16

                # ---------- main loop: outT[o,t] = sum_c w[o,c]*eegT[c,t] --
                # Stationary lhsT = wT [C=128, O] per batch; rhs streams eeg
                # at 1 col/cycle (bf16).  Pair-packing: within each half
                # batch, group 8h+j lands on partitions 0:64 and group
                # 8h+4+j on partitions 64:128 of the same PSUM bank, so one
                # [128, 512] copy drains two groups.  o_sb2 row o (o<64)
                # holds t in [8h*512, 8h*512+2048); row 64+o the next 2048.
                # gpsimd cannot access PSUM: copies alternate scalar/vector
                copy_engines = (nc.scalar, nc.vector)
                ncp = 0
                for b in range(BPC):
                    wt_b = wt_bf[:, b * O:(b + 1) * O]
                    o_sb2 = osb.tile([128, T // 2], BF16, tag="osb")
                    for h in range(2):
                        tiles = []
                        for j in range(QUAD):
                            g = HGRP * h + j
                            ps = opp.tile([128, TGRP], F32, tag="pp")
                            nc.tensor.matmul(
                                out=ps[0:O, :],
                                lhsT=wt_b,
                                rhs=e_tiles[b][:, g * TGRP:(g + 1) * TGRP],
                                start=True, stop=True,
                            )
                            tiles.append(ps)
                        for j in range(QUAD):
                            g = HGRP * h + QUAD + j
                            nc.tensor.matmul(
                                out=tiles[j][O:128, :],
                                lhsT=wt_b,
                                rhs=e_tiles[b][:, g * TGRP:(g + 1) * TGRP],
                                start=True, stop=True,
                            )
                        for j in range(QUAD):
                            dst = o_sb2[:, (QUAD * h + j) * TGRP:
                                        (QUAD * h + j + 1) * TGRP]
                            eng = copy_engines[ncp % 2]
                            ncp += 1
                            if eng is nc.scalar:
                                eng.copy(out=dst, in_=tiles[j])
                            else:
                                eng.tensor_copy(out=dst, in_=tiles[j])
                        # drain the two contiguous 2048-col quarters of this
                        # half: lower partitions then upper partitions.
                        HT = T // 4
                        nc.sync.dma_start(
                            out=outT[b][:, (2 * h) * HT:(2 * h + 1) * HT],
                            in_=o_sb2[0:O, h * HT:(h + 1) * HT],
                        )
                        nc.sync.dma_start(
                            out=outT[b][:, (2 * h + 1) * HT:(2 * h + 2) * HT],
                            in_=o_sb2[O:128, h * HT:(h + 1) * HT],
                        )


_NC_CACHE = None


def _get_nc():
    global _NC_CACHE
    if _NC_CACHE is None:
        _NC_CACHE = _build_nc()
    return _NC_CACHE


def _make_in_maps(eeg, positions, heads):
    misc, ht4 = _host_constants(np.asarray(heads, dtype=np.float32))
    eeg = np.asarray(eeg, dtype=np.float32)
    positions = np.asarray(positions, dtype=np.float32)
    in_maps = []
    for core in range(N_CORES):
        sl = slice(core * BPC, (core + 1) * BPC)
        eegT = np.ascontiguousarray(
            eeg[sl].transpose(0, 2, 1)).astype(BF16_NP)
        in_maps.append({
            "eegT": eegT,
            "posr": _pos_row(positions[sl]),
            "ht4": ht4,
            "misc": misc,
        })
    return in_maps


def kernel(eeg, positions, heads, sub=None, **_unused):
    from concourse.bass_utils import run_bass_kernel_spmd

    nc = _get_nc()
    in_maps = _make_in_maps(eeg, positions, heads)
    res = run_bass_kernel_spmd(nc, in_maps, list(range(N_CORES)))
    out = np.concatenate(
        [
            np.asarray(res.results[c]["outT"], dtype=np.float32).transpose(0, 2, 1)
            for c in range(N_CORES)
        ],
        axis=0,
    )
    return out
